# revision 1
# baseline (speedup 1.0000x reference)
"""Trainium2 Bass kernel for nn_Attention_6313601925220 (sparse_attention).

Reference computation (per (b,h) head; K == Q):
    QR = rope(Q)                      # interleaved-pair RoPE, phases = t * freqs[n]
    scores = tril(QR @ QR^T, k=-1)    # strictly causal, NO softmax
    out = scores @ V

Because there is no softmax, the strictly-causal masked product is linear and
is computed with the chunked linear-attention prefix scan:
    P_i = sum_{j<i} QR_j^T V_j                  # [N, DV] running state (PSUM, f32)
    out_i = QR_i @ P_i + tril_strict(QR_i QR_i^T) @ V_i
This is ~8x fewer FLOPs than the dense T x T score matrix (headroom=8).

Implementation notes:
  - bf16 compute on TensorE (1 cyc/row vs 4 for f32); f32 accumulation in PSUM.
  - RoPE: a = Q*cos and b = pairswap(Q)*signed_sin both run on GPSIMD (the
    pair swap is a reversed-stride access pattern, validated on HW); the add
    alternates DVE/GPSIMD by chunk parity. cos/signed-sin tables come from
    the host (computed from the freqs input).
  - P state accumulates in a persistent PSUM bank per head with a long-open
    accumulation group (HW-validated). start_tensor_calc=True clears
    has_written for the WHOLE 2KB psum bank, so only the first matmul
    touching a multi-region bank may set it.
  - Outputs accumulate 8 chunks per PSUM bank before one big evacuation;
    evacuation work is spread across ScalarE/VectorE by parity.
  - DRAM traffic is staged through SBUF in quarter-head DMA pieces,
    need-ordered so compute starts as soon as the first pieces land.

Sharding: B*NH = 32 heads, 4 heads per core across 8 cores; heads are fully
independent - no collectives.
"""

import os
import math

os.environ.setdefault("MYCRO_LOCAL_CACHE", "1")

import numpy as np
import ml_dtypes

from contextlib import ExitStack

import concourse.bass as bass
import concourse.tile as tile
from concourse import bacc, mybir
from concourse.bass_utils import run_bass_kernel_spmd

# Problem shapes (hardcoded per spec)
B, NH, T, N, DV = 2, 16, 2048, 256, 64
NCORES = 8
BH = B * NH              # 32 heads total
HPC = BH // NCORES       # 4 heads per core
TH = T * HPC             # 8192 rows of (t) per core
CH = 128                 # chunk length along t
NCH = T // CH            # 16 chunks per head

F32 = mybir.dt.float32
BF16 = mybir.dt.bfloat16
NPBF16 = ml_dtypes.bfloat16


def _build_nc():
    nc = bacc.Bacc(None, target_bir_lowering=False)

    q_d = nc.dram_tensor("q", [TH, N], BF16, kind="ExternalInput")
    v_d = nc.dram_tensor("v", [TH, DV], BF16, kind="ExternalInput")
    c_d = nc.dram_tensor("ctab", [T, N], BF16, kind="ExternalInput")   # cos table
    s_d = nc.dram_tensor("stab", [T, N], BF16, kind="ExternalInput")   # signed sin
    o_d = nc.dram_tensor("out", [TH, DV], BF16, kind="ExternalOutput")

    ident_d = nc.inline_tensor(np.eye(128).astype(NPBF16), "ident_c")
    # ST layout is [s, tq]; keep strictly-causal entries s < tq -> strict upper
    mask_d = nc.inline_tensor(np.triu(np.ones((128, 128)), k=1).astype(NPBF16),
                              "mask_c")

    PIECES = 4
    CPP = NCH // PIECES          # chunks per load piece
    OG = 8                       # chunks per out-psum group

    with tile.TileContext(nc) as tc, ExitStack() as ctx:
        consts = ctx.enter_context(tc.tile_pool(name="consts", bufs=1))
        rope = ctx.enter_context(tc.tile_pool(name="rope", bufs=8))
        qrtp = ctx.enter_context(tc.tile_pool(name="qrt", bufs=4))
        stp = ctx.enter_context(tc.tile_pool(name="st", bufs=4))
        pp = ctx.enter_context(tc.tile_pool(name="pst", bufs=10))
        ps_t = ctx.enter_context(tc.tile_pool(name="ps_t", bufs=2, space="PSUM"))
        ps_s = ctx.enter_context(tc.tile_pool(name="ps_s", bufs=2, space="PSUM"))
        ps_o = ctx.enter_context(tc.tile_pool(name="ps_o", bufs=1, space="PSUM"))
        ps_p = ctx.enter_context(tc.tile_pool(name="ps_p", bufs=1, space="PSUM"))

        ident = consts.tile([128, 128], BF16, tag="ident")
        nc.sync.dma_start(ident[:, :], ident_d[:, :])
        mask = consts.tile([128, 128], BF16, tag="mask")
        nc.sync.dma_start(mask[:, :], mask_d[:, :])

        # Piece-split staged loads: tensor X becomes PIECES tiles of
        # [128, CPP*cols]; piece p's column block c holds rows of chunk
        # p*CPP + c (so compute can start as soon as piece 0 lands).
        def declare(tag, cols):
            return [consts.tile([128, CPP * cols], BF16, tag=f"{tag}_{p}",
                                name=f"{tag}_{p}")
                    for p in range(PIECES)]

        def load_piece(tiles, p, dram, cols, row0):
            rows = slice(row0 + p * CPP * 128, row0 + (p + 1) * CPP * 128)
            nc.sync.dma_start(
                tiles[p][:, :].rearrange("p (c n) -> p c n", c=CPP),
                dram[rows, :].rearrange("(c p) n -> p c n", p=128))

        ctab = declare("ctab", N)
        stab = declare("stab", N)
        qsb = [declare(f"q{h}", N) for h in range(HPC)]
        vsb = [declare(f"v{h}", DV) for h in range(HPC)]
        osb = [consts.tile([128, NCH * DV], BF16, tag=f"o{h}", name=f"osb{h}")
               for h in range(HPC)]

        # need-ordered loads: pair 0 tensors piece by piece, then pair 1
        for p in range(PIECES):
            load_piece(ctab, p, c_d[:, :], N, 0)
            load_piece(stab, p, s_d[:, :], N, 0)
            for h in (0, 1):
                load_piece(qsb[h], p, q_d[:, :], N, h * T)
                load_piece(vsb[h], p, v_d[:, :], DV, h * T)
        for p in range(PIECES):
            for h in (2, 3):
                load_piece(qsb[h], p, q_d[:, :], N, h * T)
                load_piece(vsb[h], p, v_d[:, :], DV, h * T)

        def sl(tiles, i, cols):
            return tiles[i // CPP][:, (i % CPP) * cols:(i % CPP + 1) * cols]

        p_sb = [None] * HPC

        for hp in range(HPC // 2):
          # Two heads interleaved per pass; per-head P accumulators live in
          # PSUM with a long-open accumulation group (HW-validated pattern).
          p_ps_pair = [
              ps_p.tile([128, 2 * DV], F32, tag=f"pps{k}", name=f"pps{k}_{hp}")
              for k in range(2)
          ]
          o8_cur = [None, None]
          for i in range(NCH):
            for k in range(2):
                h = hp * 2 + k
                first = i == 0
                last = i == NCH - 1
                qi = sl(qsb[h], i, N)
                vi = sl(vsb[h], i, DV)
                ci = sl(ctab, i, N)
                si = sl(stab, i, N)

                # RoPE: a = q*cos (Pool), b = pairswap(q)*ssin (Pool),
                # qr = a + b (DVE/Pool alternating)
                a_t = rope.tile([CH, N], BF16, tag="ra")
                nc.gpsimd.tensor_mul(a_t[:, :], qi, ci)
                b_t = rope.tile([CH, N], BF16, tag="rb")
                q_sw = qi.rearrange("p (a b) -> p a b", b=2)[:, :, ::-1]
                nc.gpsimd.tensor_mul(
                    b_t[:, :].rearrange("p (a b) -> p a b", b=2), q_sw,
                    si.rearrange("p (a b) -> p a b", b=2))
                qr = rope.tile([CH, N], BF16, tag="qr")
                if i % 2 == 0:
                    nc.gpsimd.tensor_add(qr[:, :], a_t[:, :], b_t[:, :])
                else:
                    nc.vector.tensor_add(qr[:, :], a_t[:, :], b_t[:, :])

                # QRT = transpose(qr) halves (bf16 psum, no accumulation)
                qrt_ps = ps_t.tile([128, 256], BF16, tag="qrt_ps")
                for half in (slice(0, 128), slice(128, 256)):
                    nc.tensor.matmul(qrt_ps[:, half], lhsT=qr[:, half],
                                     rhs=ident[:, :], is_transpose=True,
                                     start=True, stop=True)
                qrt = qrtp.tile([128, 256], BF16, tag="qrt")
                if i % 4 == 3 and i < 8:
                    nc.vector.tensor_copy(qrt[:, :], qrt_ps[:, :])
                else:
                    nc.scalar.copy(qrt[:, :], qrt_ps[:, :])

                # Intra-chunk scores ST[s, tq] = sum_n QRT[n,s] QRT[n,tq]
                st_ps = ps_s.tile([128, 128], F32, tag="st_ps")
                nc.tensor.matmul(st_ps[:, :], lhsT=qrt[:, 0:128],
                                 rhs=qrt[:, 0:128], start=True, stop=False)
                nc.tensor.matmul(st_ps[:, :], lhsT=qrt[:, 128:256],
                                 rhs=qrt[:, 128:256], start=False, stop=True)
                st_sb = stp.tile([128, 128], BF16, tag="st_sb")
                nc.vector.tensor_mul(st_sb[:, :], st_ps[:, :], mask[:, :])

                # out_i = ST^T @ V (intra) + QR_i @ P_prev (inter), grouped
                # OG chunks per PSUM tile, one evacuation per group
                if i % OG == 0:
                    o8_cur[k] = ps_o.tile([128, OG * DV], F32, tag=f"o8_{k}",
                                          name=f"o8_{k}_{hp}_{i}")
                o_ps = o8_cur[k][:, (i % OG) * DV:(i % OG + 1) * DV]
                nc.tensor.matmul(o_ps, lhsT=st_sb[:, :], rhs=vi,
                                 start=True, stop=first)
                if not first:
                    pv = p_sb[h]
                    nc.tensor.matmul(o_ps, lhsT=qrt[:, 0:128], rhs=pv[:, 0:DV],
                                     start=False, stop=False, skip_group_check=True)
                    nc.tensor.matmul(o_ps, lhsT=qrt[:, 128:256],
                                     rhs=pv[:, DV:2 * DV],
                                     start=False, stop=True, skip_group_check=True)
                if i % OG == OG - 1:
                    g = i // OG
                    nc.scalar.copy(osb[h][:, g * OG * DV:(g + 1) * OG * DV],
                                   o8_cur[k][:, :])

                # P += QR_i^T @ V_i (accumulate in PSUM, group stays open)
                # start=True clears has_written for the WHOLE 2KB psum bank,
                # so only the very first matmul touching this bank may set it;
                # later first-writes to still-cleared elements overwrite anyway.
                for lo, nsl in ((0, slice(0, 128)), (1, slice(128, 256))):
                    reg = p_ps_pair[k][:, lo * DV:(lo + 1) * DV]
                    nc.tensor.matmul(reg, lhsT=qr[:, nsl], rhs=vi,
                                     start=(first and lo == 0), stop=last,
                                     skip_group_check=True)
                if not last:
                    p_new = pp.tile([128, 2 * DV], BF16, tag="p")
                    if i % 2 == 0:
                        nc.vector.tensor_copy(p_new[:, :], p_ps_pair[k][:, :])
                    else:
                        nc.scalar.copy(p_new[:, :], p_ps_pair[k][:, :])
                    p_sb[h] = p_new
                if i == NCH // 2 - 1 or last:
                    hw = NCH // 2
                    blk = slice(0, hw * DV) if i < hw else slice(hw * DV, NCH * DV)
                    rows_half = slice(h * T + (0 if i < hw else T // 2),
                                      h * T + (T // 2 if i < hw else T))
                    nc.sync.dma_start(
                        o_d[rows_half, :].rearrange("(c p) n -> p c n", p=128),
                        osb[h][:, blk].rearrange("p (c n) -> p c n", c=hw))

    nc.finalize()
    return nc


_NC = None


def _get_nc():
    global _NC
    if _NC is None:
        _NC = _build_nc()
    return _NC


def _host_tables(freqs):
    """cos/sin tables [T, N] from freqs [1,1,1,N] (shared across heads)."""
    f = np.asarray(freqs, dtype=np.float32).reshape(N)
    t = np.arange(T, dtype=np.float32).reshape(T, 1)
    ang = np.mod(t * f.reshape(1, N), 1.0).astype(np.float32) * np.float32(2.0 * math.pi)
    cos = np.cos(ang).astype(np.float32)
    sin = np.sin(ang).astype(np.float32)
    # signed sin: QR[2i] = q[2i]*cos[2i] - q[2i+1]*sin[2i]
    #             QR[2i+1] = q[2i+1]*cos[2i+1] + q[2i]*sin[2i+1]
    ssin = sin.copy()
    ssin[:, 0::2] *= -1.0
    return cos, ssin


def _run(inputs, trace=False, trace_kwargs=None):
    Q = np.ascontiguousarray(np.asarray(inputs["Q"], dtype=np.float32))
    V = np.ascontiguousarray(np.asarray(inputs["V"], dtype=np.float32))
    cos, ssin = _host_tables(inputs["freqs"])

    Qf = Q.reshape(BH, T, N)
    Vf = V.reshape(BH, T, DV)

    q_b = Qf.astype(NPBF16)
    v_b = Vf.astype(NPBF16)
    c_b = cos.astype(NPBF16)
    s_b = ssin.astype(NPBF16)

    in_maps = []
    for c in range(NCORES):
        hs = slice(c * HPC, (c + 1) * HPC)
        in_maps.append({
            "q": np.ascontiguousarray(q_b[hs].reshape(TH, N)),
            "v": np.ascontiguousarray(v_b[hs].reshape(TH, DV)),
            "ctab": c_b,
            "stab": s_b,
        })

    nc = _get_nc()
    kw = {}
    if trace:
        kw = dict(trace=True, trace_kwargs=trace_kwargs or {})
    res = run_bass_kernel_spmd(nc, in_maps, core_ids=list(range(NCORES)), **kw)

    out = np.empty((BH, T, DV), dtype=np.float32)
    for c in range(NCORES):
        out[c * HPC:(c + 1) * HPC] = res.results[c]["out"].reshape(HPC, T, DV)
    return out.reshape(B, NH, T, DV), res


def kernel(**inputs):
    out, _ = _run(inputs, trace=False)
    return out



# revision 6
# speedup vs baseline: 1.0155x; 1.0155x over previous
"""Trainium2 Bass kernel for nn_Attention_6313601925220 (sparse_attention).

Reference computation (per (b,h) head; K == Q):
    QR = rope(Q)                      # interleaved-pair RoPE, phases = t * freqs[n]
    scores = tril(QR @ QR^T, k=-1)    # strictly causal, NO softmax
    out = scores @ V

No softmax => the strictly-causal masked product is linear; computed with the
chunked linear-attention prefix scan:
    P_i = sum_{j<i} QR_j^T V_j                  # [N, DV] running state (PSUM, f32)
    out_i = QR_i @ P_i + tril_strict(QR_i QR_i^T) @ V_i

v2 design (cost-model driven):
  - RoPE in even/odd-split form: the host permutes Q's feature axis to
    [even | odd] halves; freqs are pair-quantized (floor(i/2)*2, per the
    reference's _get_freqs), so cos/sin tables collapse to half width:
        qrE = qE*c - qO*s ; qrO = qO*c + qE*s      (c,s = pair tables)
    6 ops of [128, G*128] per G-chunk group, all eligible for DVE 2x mode.
    The E/O relabeling is a global permutation of the contraction axis n, so
    scores and P are unchanged as long as it is applied consistently.
  - Engine budget (per 64 head-chunks): PE 832 rows/chunk (transposes 256,
    ST 256, intra 64, inter 128, P-update 128) ~= 22.2us.  Elementwise split:
    DVE = rope (most slots), Pool = rope leftovers + mask-evac + P-evac
    (pair-combined, [128,256] each), Act = qrt evac (batched x4 chunks,
    [128,1024]) + out evac ([128,512] per 8 chunks).
  - DMA: per-instruction trigger cost dominates (~500-790ns serial on SP;
    transfers serialize at ~360GB/s when the contiguous run >= 512B).  All
    tensors are host-retiled to direct SBUF images ([128, free]) so every
    transfer runs at full descriptor width, in ~30 need-ordered triggers.
  - PSUM: qrt 2 banks (bf16, 2x2-chunk batch), ST 2 banks (f32 pair tiles),
    out accumulators 3 banks, P (both heads combined) 1 bank.  start=True
    clears has_written for a whole 2KB bank; values persist and cleared
    regions are overwritten by the next write (HW-validated in the previous
    session), which makes the shared-bank P/ST packing safe with in-order PE.

Sharding: B*NH = 32 heads, 4 heads per core across 8 cores, fully
independent - no collectives.
"""

import os
import math

os.environ.setdefault("MYCRO_LOCAL_CACHE", "1")

import numpy as np
import ml_dtypes

from contextlib import ExitStack

import concourse.bass as bass
import concourse.tile as tile
from concourse import bacc, mybir
from concourse.bass_utils import run_bass_kernel_spmd

# Problem shapes (hardcoded per spec)
B, NH, T, N, DV = 2, 16, 2048, 256, 64
NCORES = 8
BH = B * NH              # 32 heads total
HPC = BH // NCORES       # 4 heads per core
CH = 128                 # chunk length along t
NCH = T // CH            # 16 chunks per head
NP = N // 2              # 128 rotation pairs

F32 = mybir.dt.float32
BF16 = mybir.dt.bfloat16
NPBF16 = ml_dtypes.bfloat16

# rope groups (start_chunk, n_chunks) per head; pass-0 heads start finer so
# compute begins as soon as the first small DMA pieces land.
GROUPS_P0 = [(0, 2), (2, 2), (4, 4), (8, 8)]
GROUPS_P1 = [(0, 8), (8, 8)]


def _build_nc():
    nc = bacc.Bacc(None, target_bir_lowering=False)

    q_d = nc.dram_tensor("q", [128, HPC * NCH * N], BF16, kind="ExternalInput")
    v_d = nc.dram_tensor("v", [128, HPC * NCH * DV], BF16, kind="ExternalInput")
    c_d = nc.dram_tensor("ctab", [128, NCH * NP], BF16, kind="ExternalInput")
    s_d = nc.dram_tensor("stab", [128, NCH * NP], BF16, kind="ExternalInput")
    o_d = nc.dram_tensor("out", [128, HPC * NCH * DV], BF16, kind="ExternalOutput")

    ident_d = nc.inline_tensor(np.eye(128).astype(NPBF16), "ident_c")
    # ST layout is [s, tq]; keep strictly-causal entries s < tq -> strict
    # upper; tiled x2 for the pair (two heads) evacuation.
    mask2_d = nc.inline_tensor(
        np.tile(np.triu(np.ones((128, 128)), k=1), (1, 2)).astype(NPBF16), "mask2_c")

    with tile.TileContext(nc) as tc, ExitStack() as ctx:
        consts = ctx.enter_context(tc.tile_pool(name="consts", bufs=1))
        ropep = ctx.enter_context(tc.tile_pool(name="rope", bufs=10))
        qrp = ctx.enter_context(tc.tile_pool(name="qr", bufs=6))
        qrtp = ctx.enter_context(tc.tile_pool(name="qrt", bufs=3))
        stp = ctx.enter_context(tc.tile_pool(name="stsb", bufs=4))
        pp = ctx.enter_context(tc.tile_pool(name="psb", bufs=4))
        ps_qrt = ctx.enter_context(tc.tile_pool(name="ps_qrt", bufs=2, space="PSUM"))
        ps_st = ctx.enter_context(tc.tile_pool(name="ps_st", bufs=2, space="PSUM"))
        ps_o = ctx.enter_context(tc.tile_pool(name="ps_o", bufs=1, space="PSUM"))
        ps_p = ctx.enter_context(tc.tile_pool(name="ps_p", bufs=1, space="PSUM"))

        ident = consts.tile([128, 128], BF16, tag="ident")
        mask2 = consts.tile([128, 256], BF16, tag="mask2")

        qsb = [consts.tile([128, NCH * N], BF16, tag=f"q{h}", name=f"q{h}")
               for h in range(HPC)]
        vsb = [consts.tile([128, NCH * DV], BF16, tag=f"v{h}", name=f"v{h}")
               for h in range(HPC)]
        ctab = consts.tile([128, NCH * NP], BF16, tag="ctab")
        stab = consts.tile([128, NCH * NP], BF16, tag="stab")
        osb = consts.tile([128, HPC * NCH * DV], BF16, tag="osb")

        def load_q(h, c0, cl):
            lo, hi = (h * NCH + c0) * N, (h * NCH + c0 + cl) * N
            nc.sync.dma_start(qsb[h][:, c0 * N:(c0 + cl) * N], q_d[:, lo:hi])

        def load_v(h, c0, cl):
            lo, hi = (h * NCH + c0) * DV, (h * NCH + c0 + cl) * DV
            nc.sync.dma_start(vsb[h][:, c0 * DV:(c0 + cl) * DV], v_d[:, lo:hi])

        def load_tab(c0, cl):
            nc.sync.dma_start(ctab[:, c0 * NP:(c0 + cl) * NP],
                              c_d[:, c0 * NP:(c0 + cl) * NP])
            nc.sync.dma_start(stab[:, c0 * NP:(c0 + cl) * NP],
                              s_d[:, c0 * NP:(c0 + cl) * NP])

        # need-ordered loads (SP serial): early pieces small, later big.
        load_tab(0, 2)
        load_q(0, 0, 2)
        load_q(1, 0, 2)
        nc.sync.dma_start(ident[:, :], ident_d[:, :])
        load_tab(2, 2)
        load_q(0, 2, 2)
        load_q(1, 2, 2)
        load_v(0, 0, 8)
        load_v(1, 0, 8)
        nc.sync.dma_start(mask2[:, :], mask2_d[:, :])
        load_tab(4, 4)
        load_q(0, 4, 4)
        load_q(1, 4, 4)
        load_tab(8, 8)
        load_q(0, 8, 8)
        load_q(1, 8, 8)
        load_v(0, 8, 8)
        load_v(1, 8, 8)
        for h in (2, 3):
            load_q(h, 0, 8)
            load_v(h, 0, 8)
        for h in (2, 3):
            load_q(h, 8, 8)
            load_v(h, 8, 8)

        ctv = ctab[:, :].rearrange("p (c k) -> p c k", c=NCH)
        stv = stab[:, :].rearrange("p (c k) -> p c k", c=NCH)

        # rope engine schedule: 6 op slots per group
        #   [m1=qE*c, m2=qO*s, m3=qO*c, m4=qE*s, qrE=m1-m2, qrO=m3+m4]
        # DVE is cheapest (2x mode); Pool takes ~1.25 slots on average.
        rope_ctr = [0]

        def emit_rope(h, c0, cl, qr_tile):
            g = rope_ctr[0]
            rope_ctr[0] += 1
            qv = qsb[h][:, :].rearrange("p (c n) -> p c n", c=NCH)
            qE = qv[:, c0:c0 + cl, 0:NP]
            qO = qv[:, c0:c0 + cl, NP:N]
            cv = ctv[:, c0:c0 + cl, :]
            sv = stv[:, c0:c0 + cl, :]
            qrv = qr_tile[:, :].rearrange("p (c e k) -> p c e k", c=cl, e=2)
            qrE = qrv[:, :, 0, :]
            qrO = qrv[:, :, 1, :]

            def mt(tag):
                t = ropep.tile([128, cl * NP], BF16, tag=tag)
                return t[:, :].rearrange("p (c k) -> p c k", c=cl)

            m1, m2, m3, m4 = mt("m1"), mt("m2"), mt("m3"), mt("m4")
            # GPSIMD cannot touch PSUM, so Pool only ever does rope; give it
            # ~4 of the 6 slots (every 8th group one back to DVE).
            dve_m3 = (g % 8 == 7)
            nc.gpsimd.tensor_mul(m1, qE, cv)
            nc.vector.tensor_mul(m2, qO, sv)
            if dve_m3:
                nc.vector.tensor_mul(m3, qO, cv)
            else:
                nc.gpsimd.tensor_mul(m3, qO, cv)
            nc.gpsimd.tensor_mul(m4, qE, sv)
            nc.vector.tensor_sub(qrE, m1, m2)
            nc.gpsimd.tensor_add(qrO, m3, m4)

        # per pass: heads (2p, 2p+1) chunk-locked
        for pass_i in (0, 1):
            heads = (2 * pass_i, 2 * pass_i + 1)
            groups = GROUPS_P0 if pass_i == 0 else GROUPS_P1
            # rope group emission points: block index (2-chunk) -> groups
            emit_at = {}
            for gi, (c0, cl) in enumerate(groups):
                blk = 0 if c0 < 6 else max(0, c0 // 2 - 2)
                emit_at.setdefault(blk, []).append((c0, cl))

            qr_tiles = {}      # (k, group_first_chunk) -> (tile, c0, cl)

            def emit_rope_groups(blk):
                for (c0, cl) in emit_at.get(blk, []):
                    for k, h in enumerate(heads):
                        t = qrp.tile([128, cl * N], BF16, tag=f"qr{k}",
                                     name=f"qr_{pass_i}_{k}_{c0}")
                        emit_rope(h, c0, cl, t)
                        for c in range(c0, c0 + cl):
                            qr_tiles[(k, c)] = (t, c0)

            def qr_slice(k, c, half):
                t, c0 = qr_tiles[(k, c)]
                v = t[:, :].rearrange("p (c e k) -> p c e k", c=(t.shape[1] // N), e=2)
                return v[:, c - c0, half, :]

            # transposes of block j (chunks 2j, 2j+1, both heads) go into one
            # 2KB bf16 psum bank; one Act evac per block pair.
            qrt_sb = {}        # block -> sbuf tile [128, 1024]

            def emit_transposes(j):
                c_pair = (2 * j, 2 * j + 1)
                ps = ps_qrt.tile([128, 1024], BF16, tag="qrt_ps",
                                 name=f"qrtps_{pass_i}_{j}")
                for ci, c in enumerate(c_pair):
                    for k in range(2):
                        for half in range(2):
                            off = ((ci * 2 + k) * 2 + half) * 128
                            nc.tensor.matmul(
                                ps[:, off:off + 128], lhsT=qr_slice(k, c, half),
                                rhs=ident[:, :], is_transpose=True,
                                start=True, stop=True)
                sb = qrtp.tile([128, 1024], BF16, tag="qrt_sb",
                               name=f"qrtsb_{pass_i}_{j}")
                # alternate the evac between Act and DVE (~4/7 on Act)
                if j % 7 in (0, 2, 4, 6):
                    nc.scalar.copy(sb[:, :], ps[:, :])
                else:
                    nc.vector.tensor_copy(sb[:, :], ps[:, :])
                qrt_sb[j] = sb

            def qrt_slice(k, c, half):
                j = c // 2
                ci = c % 2
                off = ((ci * 2 + k) * 2 + half) * 128
                return qrt_sb[j][:, off:off + 128]

            p_ps = ps_p.tile([128, 256], F32, tag="pps", name=f"pps{pass_i}")
            o8 = [None, None]
            p_sb_prev = None

            emit_rope_groups(0)
            emit_transposes(0)

            for j in range(NCH // 2):          # 8 blocks of 2 chunks
                emit_rope_groups(j + 1)
                if j + 1 < NCH // 2:
                    emit_transposes(j + 1)
                for c in (2 * j, 2 * j + 1):
                    first = c == 0
                    last = c == NCH - 1
                    # ST for both heads into one f32 pair tile [128, 256]
                    st_ps = ps_st.tile([128, 256], F32, tag="st_ps",
                                       name=f"stps_{pass_i}_{c}")
                    for k in range(2):
                        sl = st_ps[:, k * 128:(k + 1) * 128]
                        nc.tensor.matmul(sl, lhsT=qrt_slice(k, c, 0),
                                         rhs=qrt_slice(k, c, 0),
                                         start=True, stop=False)
                        nc.tensor.matmul(sl, lhsT=qrt_slice(k, c, 1),
                                         rhs=qrt_slice(k, c, 1),
                                         start=False, stop=True)
                    # P += QR_c^T V_c for both heads (shared bank, long-open
                    # group; only the very first matmul of the pass starts it)
                    for k, h in enumerate(heads):
                        vi = vsb[h][:, c * DV:(c + 1) * DV]
                        for half in range(2):
                            reg = p_ps[:, k * 128 + half * 64:k * 128 + (half + 1) * 64]
                            nc.tensor.matmul(
                                reg, lhsT=qr_slice(k, c, half), rhs=vi,
                                start=(first and k == 0 and half == 0),
                                stop=last, skip_group_check=True)
                    # evacuations for chunk c: masked ST on DVE, new P on Act
                    # (GPSIMD can't read PSUM)
                    st_sb = stp.tile([128, 256], BF16, tag="st_sb",
                                     name=f"stsb_{pass_i}_{c}")
                    nc.vector.tensor_mul(st_sb[:, :], st_ps[:, :], mask2[:, :])
                    if not last:
                        p_new = pp.tile([128, 256], BF16, tag="p_sb",
                                        name=f"psb_{pass_i}_{c}")
                        nc.scalar.copy(p_new[:, :], p_ps[:, :])
                    else:
                        p_new = None
                    # out accumulation: intra + inter into per-head o8 banks
                    for k, h in enumerate(heads):
                        vi = vsb[h][:, c * DV:(c + 1) * DV]
                        if c % 8 == 0:
                            o8[k] = ps_o.tile([128, 512], F32, tag=f"o8_{k}",
                                              name=f"o8_{pass_i}_{k}_{c}")
                        o_sl = o8[k][:, (c % 8) * DV:(c % 8 + 1) * DV]
                        nc.tensor.matmul(o_sl, lhsT=st_sb[:, k * 128:(k + 1) * 128],
                                         rhs=vi, start=True, stop=first)
                        if not first:
                            for half in range(2):
                                pv = p_sb_prev[:, k * 128 + half * 64:
                                               k * 128 + (half + 1) * 64]
                                nc.tensor.matmul(
                                    o_sl, lhsT=qrt_slice(k, c, half), rhs=pv,
                                    start=False, stop=(half == 1),
                                    skip_group_check=True)
                        if c % 8 == 7:
                            g8 = c // 8
                            base = (h * NCH + g8 * 8) * DV
                            nc.scalar.copy(osb[:, base:base + 512], o8[k][:, :])
                            nc.sync.dma_start(o_d[:, base:base + 512],
                                              osb[:, base:base + 512])
                    p_sb_prev = p_new

    nc.finalize()
    return nc


_NC = None


def _get_nc():
    global _NC
    if _NC is None:
        _NC = _build_nc()
    return _NC


def _host_prep(Q, V, freqs):
    """Host-side retiling to direct SBUF images.

    - Q feature axis permuted to [even | odd] halves (global relabeling of the
      contraction axis; scores/P invariant).
    - cos/sin pair tables [T, 128] (freqs are pair-quantized in the reference:
      floor(i/2)*2, so cos/sin agree within each (2i, 2i+1) pair).
    - every tensor stored as [128, free] so each DMA row is one contiguous
      descriptor run.
    """
    Qf = np.asarray(Q, dtype=np.float32).reshape(BH, T, N)
    Vf = np.asarray(V, dtype=np.float32).reshape(BH, T, DV)
    f = np.asarray(freqs, dtype=np.float32).reshape(N)

    t = np.arange(T, dtype=np.float32).reshape(T, 1)
    ang = np.mod(t * f.reshape(1, N), 1.0).astype(np.float32) * np.float32(2.0 * math.pi)
    ce = np.cos(ang[:, 0::2]).astype(NPBF16)     # [T, 128]
    se = np.sin(ang[:, 0::2]).astype(NPBF16)
    ctab = ce.reshape(NCH, CH, NP).transpose(1, 0, 2).reshape(128, NCH * NP)
    stab = se.reshape(NCH, CH, NP).transpose(1, 0, 2).reshape(128, NCH * NP)

    perm = np.concatenate([np.arange(0, N, 2), np.arange(1, N, 2)])
    Qp = Qf[:, :, perm].astype(NPBF16)           # [BH, T, N] -> E|O halves
    Vb = Vf.astype(NPBF16)

    q_cores = []
    v_cores = []
    for c in range(NCORES):
        hs = slice(c * HPC, (c + 1) * HPC)
        qc = Qp[hs].reshape(HPC, NCH, CH, N).transpose(2, 0, 1, 3).reshape(
            128, HPC * NCH * N)
        vc = Vb[hs].reshape(HPC, NCH, CH, DV).transpose(2, 0, 1, 3).reshape(
            128, HPC * NCH * DV)
        q_cores.append(np.ascontiguousarray(qc))
        v_cores.append(np.ascontiguousarray(vc))
    return q_cores, v_cores, np.ascontiguousarray(ctab), np.ascontiguousarray(stab)


def _run(inputs, trace=False, trace_kwargs=None):
    q_cores, v_cores, ctab, stab = _host_prep(
        inputs["Q"], inputs["V"], inputs["freqs"])

    in_maps = []
    for c in range(NCORES):
        in_maps.append({
            "q": q_cores[c],
            "v": v_cores[c],
            "ctab": ctab,
            "stab": stab,
        })

    nc = _get_nc()
    kw = {}
    if trace:
        kw = dict(trace=True, trace_kwargs=trace_kwargs or {})
    res = run_bass_kernel_spmd(nc, in_maps, core_ids=list(range(NCORES)), **kw)

    out = np.empty((BH, T, DV), dtype=np.float32)
    for c in range(NCORES):
        oc = res.results[c]["out"].astype(np.float32)        # [128, HPC*NCH*DV]
        oc = oc.reshape(128, HPC, NCH, DV).transpose(1, 2, 0, 3)
        out[c * HPC:(c + 1) * HPC] = oc.reshape(HPC, T, DV)
    return out.reshape(B, NH, T, DV), res


def kernel(**inputs):
    out, _ = _run(inputs, trace=False)
    return out
